# revision 1
# baseline (speedup 1.0000x reference)
"""Trainium2 Bass kernel for a batched-ensemble MLP (nn_BMLP_773094113632).

Network per ensemble member e (64 members):
    u = silu(x @ w0 + b0); u = silu(u @ w1 + b1); u = silu(u @ w2 + b2)
    y = u @ wl + bl
Shapes: x [64, 4096, 16], hidden 256, out 1.

Strategy: shard the 64 ensemble members across 8 NeuronCores (8 members per
core, embarrassingly parallel).  On each core, activations live in SBUF as
[hidden(partition), points(free)] tiles; each layer is a chain of PE matmuls
(weights stationary, float32r = tf32-class) accumulated in PSUM, with the
SiLU + bias evacuating PSUM->SBUF on the scalar engine.  Layer 0 folds its
bias into the matmul via an appended ones-row and packs its K=17 matmuls
4-per-PE-pass with tile_position row groups (x^T is host-replicated into all
four row groups).  The final [256 -> 1] layer runs as M=1 matmuls and its
bias is added by the vector engine during PSUM evacuation; it is
software-pipelined one member behind so the scalar engine never starves.
End-to-end output error vs the fp32 reference is ~2.4e-4.
"""

import sys

sys.path.insert(0, "/opt/trn_rl_repo")

import numpy as np

import concourse.tile as tile
from concourse import bacc, mybir

F32 = mybir.dt.float32
F32R = mybir.dt.float32r
AFT = mybir.ActivationFunctionType

E = 64  # ensemble members
NPTS = 4096
INDIM = 16
HID = 256
N_CORES = 8
EPC = E // N_CORES  # members per core
KQ = INDIM + 1  # layer-0 contraction: indim + ones row (bias fold)
MMQ = 512  # matmul N chunk (one fp32 PSUM bank)


def build(
    reps: int = 1,
    dtype=F32R,
    hw_loop: bool = False,
    passes: int = 1,
    group: int = 2048,
    ps_bufs: int | None = None,
):
    GRP = group  # ACT group width; GRP/512 PSUM banks per group
    NG = NPTS // GRP  # groups per (layer, mt)
    GQ = GRP // MMQ  # matmul chunks per group
    if ps_bufs is None:
        ps_bufs = (8 * MMQ) // GRP  # fill all 8 PSUM banks

    nc = bacc.Bacc("TRN2", target_bir_lowering=False, debug=False)

    xt_d = nc.dram_tensor("xt", [EPC, 128, NPTS], dtype, kind="ExternalInput").ap()
    w0_d = nc.dram_tensor("w0p", [EPC, 128, HID], dtype, kind="ExternalInput").ap()
    w1_d = nc.dram_tensor("w1p", [EPC, 128, 512], dtype, kind="ExternalInput").ap()
    w2_d = nc.dram_tensor("w2p", [EPC, 128, 512], dtype, kind="ExternalInput").ap()
    wl_d = nc.dram_tensor("wlp", [128, 2 * EPC], dtype, kind="ExternalInput").ap()
    bias_d = nc.dram_tensor("biasp", [128, 4 * EPC], F32, kind="ExternalInput").ap()
    bl_d = nc.dram_tensor("blp", [1, EPC], F32, kind="ExternalInput").ap()
    y_d = nc.dram_tensor("y", [EPC, NPTS], F32, kind="ExternalOutput").ap()

    with tile.TileContext(nc) as tc:
        with (
            tc.tile_pool(name="const", bufs=1) as const_pool,
            tc.tile_pool(name="w12", bufs=6) as w12_pool,
            tc.tile_pool(name="xt", bufs=2 * NG) as xt_pool,
            tc.tile_pool(name="u", bufs=6 * NG) as u_pool,
            tc.tile_pool(name="y", bufs=2 * NG) as y_pool,
            tc.tile_pool(name="ps", bufs=ps_bufs, space="PSUM") as ps_pool,
        ):
            wl_sb = const_pool.tile([128, 2 * EPC], dtype, tag="wl")
            nc.sync.dma_start(wl_sb[:], wl_d)
            bias_sb = const_pool.tile([128, 4 * EPC], F32, tag="bias")
            nc.sync.dma_start(bias_sb[:], bias_d)
            bl_sb = const_pool.tile([1, EPC], F32, tag="bl")
            nc.sync.dma_start(bl_sb[:], bl_d)

            def emit_l0(i, w0_sb, xt_sb, u, g, mt):
                ps = ps_pool.tile([128, GRP], F32, tag="ps")
                for q in range(GQ):
                    # x^T is replicated in the 4 PE row groups; chunk q uses
                    # row group q%4 so up to 4 matmuls run concurrently.
                    j = q % 4
                    rows = slice(32 * j, 32 * j + KQ)
                    nc.tensor.matmul(
                        ps[:, q * MMQ : (q + 1) * MMQ],
                        w0_sb[rows, mt * 128 : (mt + 1) * 128],
                        xt_sb[rows, q * MMQ : (q + 1) * MMQ],
                        start=True,
                        stop=True,
                        tile_position=(32 * j, 0),
                    )
                u0 = u_pool.tile([128, GRP], dtype, tag="u")
                nc.scalar.activation(u0[:], ps[:], AFT.Silu)
                u[0][mt][g] = u0

            def emit_l12(i, layer, w_sb, u, g, mt):
                ps = ps_pool.tile([128, GRP], F32, tag="ps")
                for kt in range(2):
                    for q in range(GQ):
                        nc.tensor.matmul(
                            ps[:, q * MMQ : (q + 1) * MMQ],
                            w_sb[:, kt * HID + mt * 128 : kt * HID + (mt + 1) * 128],
                            u[layer - 1][kt][g][:, q * MMQ : (q + 1) * MMQ],
                            start=(kt == 0),
                            stop=(kt == 1),
                        )
                ul = u_pool.tile([128, GRP], dtype, tag="u")
                bias_ap = bias_sb[
                    :,
                    i * 4 + (layer - 1) * 2 + mt : i * 4 + (layer - 1) * 2 + mt + 1,
                ]
                nc.scalar.activation(ul[:], ps[:], AFT.Silu, bias=bias_ap)
                u[layer][mt][g] = ul

            def emit_l3(i, u2, g):
                """Final [256->1] layer, one points-group, for member i."""
                ps = ps_pool.tile([1, GRP], F32, tag="ps")
                for kt in range(2):
                    for q in range(GQ):
                        nc.tensor.matmul(
                            ps[0:1, q * MMQ : (q + 1) * MMQ],
                            wl_sb[:, i * 2 + kt : i * 2 + kt + 1],
                            u2[kt][g][:, q * MMQ : (q + 1) * MMQ],
                            start=(kt == 0),
                            stop=(kt == 1),
                        )
                y_sb = y_pool.tile([1, GRP], F32, tag="y")
                # quarter-split evac: releases PSUM bank progress earlier
                for q in range(GQ):
                    nc.vector.tensor_scalar_add(
                        y_sb[0:1, q * MMQ : (q + 1) * MMQ],
                        ps[0:1, q * MMQ : (q + 1) * MMQ],
                        bl_sb[0:1, i : i + 1],
                    )
                nc.sync.dma_start(y_d[i : i + 1, g * GRP : (g + 1) * GRP], y_sb[:])

            def one_pass():
                # Member i's ACT-free final layer is interleaved into member
                # i+1's layer-0 stream so the scalar engine never starves.
                prev = None  # (member index, u2 tiles)
                for i in range(EPC):
                    w0_sb = w12_pool.tile([128, HID], dtype, tag="w0")
                    nc.sync.dma_start(w0_sb[:], w0_d[i])
                    w1_sb = w12_pool.tile([128, 512], dtype, tag="w12")
                    nc.sync.dma_start(w1_sb[:], w1_d[i])
                    w2_sb = w12_pool.tile([128, 512], dtype, tag="w12")
                    nc.sync.dma_start(w2_sb[:], w2_d[i])

                    # u[layer][mt][group] -> SBUF tile [128, GRP]
                    u = [[[None] * NG, [None] * NG] for _ in range(3)]

                    for g in range(NG):
                        xt_sb = xt_pool.tile([128, GRP], dtype, tag="xt")
                        # split across partition halves -> two DMA queues, so
                        # the pass-leading load isn't a single-queue latency
                        nc.sync.dma_start(
                            xt_sb[0:64, :], xt_d[i][0:64, g * GRP : (g + 1) * GRP]
                        )
                        nc.sync.dma_start(
                            xt_sb[64:128, :], xt_d[i][64:128, g * GRP : (g + 1) * GRP]
                        )
                        emit_l0(i, w0_sb, xt_sb, u, g, 0)
                        emit_l0(i, w0_sb, xt_sb, u, g, 1)
                        if prev is not None:
                            emit_l3(prev[0], prev[1], g)
                    for g in range(NG):
                        emit_l12(i, 1, w1_sb, u, g, 0)
                        emit_l12(i, 1, w1_sb, u, g, 1)
                    for g in range(NG):
                        emit_l12(i, 2, w2_sb, u, g, 0)
                        emit_l12(i, 2, w2_sb, u, g, 1)

                    # u2 is indexed [kt][g] by the final layer (kt == mt here)
                    prev = (i, u[2])

                for g in range(NG):
                    emit_l3(prev[0], prev[1], g)

            if hw_loop:
                hints = (
                    (
                        mybir.EngineType.PE,
                        mybir.EngineType.Activation,
                        mybir.EngineType.SP,
                        mybir.EngineType.DVE,
                    )
                    if hw_loop == "hints"
                    else ()
                )
                with tc.For_i(
                    0,
                    reps,
                    1,
                    staggered_reset=hw_loop == "staggered",
                    hint_engines=hints,
                ):
                    for _ in range(passes):
                        one_pass()
            else:
                for _ in range(reps):
                    one_pass()

    nc.compile()
    return nc


def pack_inputs(x, w0, b0, w1, b1, w2, b2, wl, bl):
    """Split the full-ensemble inputs into 8 per-core input maps."""
    f = np.float32
    x = np.ascontiguousarray(x, dtype=f)
    in_maps = []
    for c in range(N_CORES):
        sl = slice(c * EPC, (c + 1) * EPC)
        # x^T (+ ones row for the bias fold) replicated into the 4 PE row
        # groups so layer 0 can run 4 concurrent row-tiled matmuls.
        xt = np.zeros((EPC, 128, NPTS), f)
        w0p = np.zeros((EPC, 128, HID), f)
        for j in range(4):
            xt[:, 32 * j : 32 * j + INDIM, :] = x[sl].transpose(0, 2, 1)
            xt[:, 32 * j + INDIM, :] = 1.0
            w0p[:, 32 * j : 32 * j + INDIM, :] = w0[sl]
            w0p[:, 32 * j + INDIM, :] = b0[sl, 0]

        # [e, 256, 256] -> [e, 128(p), 2(kt)*256]
        w1p = np.ascontiguousarray(
            w1[sl].reshape(EPC, 2, 128, HID).transpose(0, 2, 1, 3).reshape(EPC, 128, 512),
            dtype=f,
        )
        w2p = np.ascontiguousarray(
            w2[sl].reshape(EPC, 2, 128, HID).transpose(0, 2, 1, 3).reshape(EPC, 128, 512),
            dtype=f,
        )
        # [e, 256, 1] -> [128(p), e*2(kt)]
        wlp = np.ascontiguousarray(
            wl[sl].reshape(EPC, 2, 128).transpose(2, 0, 1).reshape(128, 2 * EPC),
            dtype=f,
        )
        # [128(p), e*4] cols: b1 mt0, b1 mt1, b2 mt0, b2 mt1
        biasp = np.ascontiguousarray(
            np.stack(
                [b1[sl, 0, :128], b1[sl, 0, 128:], b2[sl, 0, :128], b2[sl, 0, 128:]],
                axis=1,
            )
            .transpose(2, 0, 1)
            .reshape(128, 4 * EPC),
            dtype=f,
        )
        blp = np.ascontiguousarray(bl[sl, 0, 0].reshape(1, EPC), dtype=f)
        in_maps.append(
            {
                "xt": xt,
                "w0p": w0p,
                "w1p": w1p,
                "w2p": w2p,
                "wlp": wlp,
                "biasp": biasp,
                "blp": blp,
            }
        )
    return in_maps


def make_runner(nc):
    """Compile nc once into a persistent 8-core jitted callable.

    Mirrors bass2jax.run_bass_via_pjrt's multi-core path but caches the
    compiled executable and (optionally) the device-resident inputs so
    repeated calls only pay one RPC.
    """
    import jax
    from jax.experimental.shard_map import shard_map
    from jax.sharding import Mesh, PartitionSpec

    from concourse import bass2jax

    bass2jax.install_neuronx_cc_hook()

    partition_name = nc.partition_id_tensor.name if nc.partition_id_tensor else None
    in_names, out_names, out_avals, zero_outs = [], [], [], []
    for alloc in nc.m.functions[0].allocations:
        if not isinstance(alloc, mybir.MemoryLocationSet):
            continue
        name = alloc.memorylocations[0].name
        if alloc.kind == "ExternalInput":
            if name != partition_name:
                in_names.append(name)
        elif alloc.kind == "ExternalOutput":
            out_names.append(name)
            shape = tuple(alloc.tensor_shape)
            dt = mybir.dt.np(alloc.dtype)
            out_avals.append(jax.core.ShapedArray(shape, dt))
            zero_outs.append(np.zeros(shape, dt))
    n_params = len(in_names)
    n_outs = len(out_names)
    all_names = in_names + out_names
    if partition_name is not None:
        all_names = all_names + [partition_name]
    donate = tuple(range(n_params, n_params + n_outs))

    def _body(*args):
        operands = list(args)
        if partition_name is not None:
            operands.append(bass2jax.partition_id_tensor())
        outs = bass2jax._bass_exec_p.bind(
            *operands,
            out_avals=tuple(out_avals),
            in_names=tuple(all_names),
            out_names=tuple(out_names),
            lowering_input_output_aliases=(),
            sim_require_finite=True,
            sim_require_nnan=True,
            nc=nc,
        )
        return tuple(outs)

    devices = jax.devices()[:N_CORES]
    mesh = Mesh(np.asarray(devices), ("core",))
    sharded = jax.jit(
        shard_map(
            _body,
            mesh=mesh,
            in_specs=(PartitionSpec("core"),) * (n_params + n_outs),
            out_specs=(PartitionSpec("core"),) * n_outs,
            check_rep=False,
        ),
        donate_argnums=donate,
        keep_unused=True,
    )

    state = {}

    def run(in_maps, cache_inputs=False):
        if not cache_inputs or "dev_in" not in state:
            concat_in = [
                np.concatenate([np.asarray(m[name]) for m in in_maps], axis=0)
                for name in in_names
            ]
            state["dev_in"] = [jax.device_put(a) for a in concat_in]
            for a in state["dev_in"]:
                a.block_until_ready()
        concat_zeros = [
            np.zeros((N_CORES * z.shape[0], *z.shape[1:]), z.dtype) for z in zero_outs
        ]
        out_arrs = sharded(*state["dev_in"], *concat_zeros)
        out_arrs = [np.asarray(o) for o in out_arrs]
        return [
            {
                name: out_arrs[i].reshape(N_CORES, *out_avals[i].shape)[c]
                for i, name in enumerate(out_names)
            }
            for c in range(N_CORES)
        ]

    return run


_RUNNER_CACHE = {}


def _get_runner(reps=1, hw_loop=False, passes=1, **bkw):
    key = (reps, hw_loop, passes, tuple(sorted(bkw.items())))
    if key not in _RUNNER_CACHE:
        _RUNNER_CACHE[key] = make_runner(
            build(reps, hw_loop=hw_loop, passes=passes, **bkw)
        )
    return _RUNNER_CACHE[key]


def run(in_maps, reps=1, hw_loop=False, cache_inputs=False, passes=1, **bkw):
    return _get_runner(reps, hw_loop, passes, **bkw)(in_maps, cache_inputs=cache_inputs)


def kernel(x, w0, b0, w1, b1, w2, b2, wl, bl):
    in_maps = pack_inputs(x, w0, b0, w1, b1, w2, b2, wl, bl)
    results = run(in_maps)
    y = np.concatenate([results[c]["y"] for c in range(N_CORES)], axis=0)
    return y.reshape(E, NPTS, 1).astype(np.float32)


if __name__ == "__main__":
    rng = np.random.default_rng(0)
    ins = {
        "x": rng.standard_normal((E, NPTS, INDIM), dtype=np.float32),
        "w0": rng.standard_normal((E, INDIM, HID), dtype=np.float32) * 0.25,
        "b0": rng.standard_normal((E, 1, HID), dtype=np.float32) * 0.25,
        "w1": rng.standard_normal((E, HID, HID), dtype=np.float32) * 0.06,
        "b1": rng.standard_normal((E, 1, HID), dtype=np.float32) * 0.06,
        "w2": rng.standard_normal((E, HID, HID), dtype=np.float32) * 0.06,
        "b2": rng.standard_normal((E, 1, HID), dtype=np.float32) * 0.06,
        "wl": rng.standard_normal((E, HID, 1), dtype=np.float32) * 0.06,
        "bl": rng.standard_normal((E, 1, 1), dtype=np.float32) * 0.06,
    }
    out = kernel(**ins)

    def silu(v):
        return v / (1.0 + np.exp(-v))

    u = silu(ins["x"] @ ins["w0"] + ins["b0"])
    u = silu(u @ ins["w1"] + ins["b1"])
    u = silu(u @ ins["w2"] + ins["b2"])
    ref = u @ ins["wl"] + ins["bl"]
    err = np.abs(out - ref).max() / np.abs(ref).max()
    print("self-test rel err:", err)

